# revision 16
# baseline (speedup 1.0000x reference)
"""GQA attention (B=1, S=2048, D=2048, H=32, KV=4, HD=64) on 8 TRN2 NeuronCores.

Sharding: tensor-parallel over heads. Core c owns q-heads [4c, 4c+4) and kv-head
c//2 (all four q-heads of a core share one kv head).

Per-core pipeline (single fused loop over 512-token chunks):
  1. x streamed in S-chunks via SWDGE cast-DMA (f32->bf16, token-major), then
     XBAR DMA-transposed to x^T blocks (zero PE/ACT cost).
  2. Projections token-major: stationary x^T blocks, streaming [wq|wkv] (N=384),
     one PSUM bank per 128-token block.
  3. RoPE applied token-major (cos/sin per-partition tiles, natural (2i,2i+1)
     pairing - no weight permutation), split across DVE/GpSimd, reading PSUM
     directly; Q/K then DMA-transposed to Q^T/K^T; V copied straight into the
     key-major V tile (ones column for softmax denominators).
  4. Flash-style causal attention per (head-pair, query-chunk): scores^T via
     PE row-group packing (2 heads), exp on ScalarE over [128,2x512] tiles,
     matmul N-ranges restricted to the causal band at pair granularity,
     triangle masks only on diagonal strips, PV with ones-column denominator.
  5. Normalize via DVE reciprocal + GpSimd partition_broadcast, chunked
     AllGather (last chunk split per head-pair to shorten the tail), output
     projection interleaved one chunk behind attention.
Returns out^T [256, 2048] per core; the host transposes/concatenates.
"""

import os
from contextlib import ExitStack

import numpy as np

import concourse.bass as bass
import concourse.mybir as mybir
import concourse.tile as tile
from concourse import bacc
from concourse import bass_utils

F32 = mybir.dt.float32
BF16 = mybir.dt.bfloat16
AF = mybir.ActivationFunctionType
ALU = mybir.AluOpType

S = 2048
D = 2048
HD = 64
CORES = 8
SC = 512
NSC = S // SC
NDC = D // 128
TB = 4  # 128-token blocks per chunk

_NC_CACHE = {}


def _dram3(t, row0, nrow_p, nblk, blk_stride, ncol, col0=0):
    """AP over DRAM tensor/AP t: [partition p, block b, col f] ->
    t[row0 + b*blk_stride + p, col0 + f], p<nrow_p, b<nblk, f<ncol."""
    if isinstance(t, bass.AP):
        handle, off0, row_pitch = t.tensor, t.offset, t.tensor.shape[1]
    else:
        handle, off0, row_pitch = t, 0, t.shape[1]
    return bass.AP(
        handle,
        off0 + row0 * row_pitch + col0,
        [[row_pitch, nrow_p], [blk_stride * row_pitch, nblk], [1, ncol]],
    )


def _sb(t, off, dims):
    """Multi-dim free AP over an SBUF/PSUM tile at element offset off."""
    return bass.AP(t.tensor, t.offset + off, [t.ap[0], *dims])


def build():
    if "nc" in _NC_CACHE:
        return _NC_CACHE["nc"]
    nc = bacc.Bacc(None, target_bir_lowering=False, debug=False)

    x = nc.declare_dram_parameter("x", [S, D], F32, isOutput=False)
    wq = nc.declare_dram_parameter("wq", [D, 256], F32, isOutput=False)
    wkv = nc.declare_dram_parameter("wkv", [D, 128], F32, isOutput=False)
    wo = nc.declare_dram_parameter("wo", [D, 256], F32, isOutput=False)
    fcos = nc.declare_dram_parameter("fcos", [S, 32], F32, isOutput=False)
    fsin = nc.declare_dram_parameter("fsin", [S, 32], F32, isOutput=False)
    out = nc.declare_dram_parameter("out", [256, S], F32, isOutput=True)
    dbg = os.environ.get("KDBG") == "1"
    if dbg:
        d_qt = nc.declare_dram_parameter("d_qt", [128, S], BF16, isOutput=True)
        d_kt = nc.declare_dram_parameter("d_kt", [128, S], BF16, isOutput=True)
        d_vx = nc.declare_dram_parameter("d_vx", [128, 16 * 65], BF16, isOutput=True)
        d_at = nc.declare_dram_parameter("d_at", [128, S], BF16, isOutput=True)

    with tile.TileContext(nc) as tc:
        with ExitStack() as stack:
            ent = stack.enter_context
            const = ent(tc.tile_pool(name="const", bufs=1))
            wpool = ent(tc.tile_pool(name="wpool", bufs=1))
            big = ent(tc.tile_pool(name="big", bufs=1))
            xp = ent(tc.tile_pool(name="xp", bufs=2))
            xtp = ent(tc.tile_pool(name="xtp", bufs=5))
            qs = ent(tc.tile_pool(name="qs", bufs=3))
            qkb = ent(tc.tile_pool(name="qkb", bufs=3))
            kvs = ent(tc.tile_pool(name="kvs", bufs=3))
            rp = ent(tc.tile_pool(name="rp", bufs=6))
            ptp = ent(tc.tile_pool(name="ptp", bufs=4))
            npool = ent(tc.tile_pool(name="npool", bufs=2))
            agt = ent(tc.tile_pool(name="agt", bufs=2))
            otp = ent(tc.tile_pool(name="otp", bufs=2))
            pp = ent(tc.tile_pool(name="pp", bufs=2, space="PSUM"))
            sp = ent(tc.tile_pool(name="sp", bufs=2, space="PSUM"))
            pvp = ent(tc.tile_pool(name="pvp", bufs=2, space="PSUM"))
            dram = ent(tc.tile_pool(name="dram", bufs=1, space="DRAM"))
            # ---- critical path: x chunk 0 (split per token-block) ----
            xbf0 = xp.tile([128, TB * D], BF16, name="xbf")
            for tb in range(TB):
                nc.gpsimd.dma_start(
                    out=xbf0[:, tb * D : (tb + 1) * D],
                    in_=x[tb * 128 : tb * 128 + 128, :],
                )
            # weights: [wq | wkv] interleaved per 128-row D-block
            wqkvb = wpool.tile([128, NDC * 384], BF16)
            nc.gpsimd.dma_start(
                out=_sb(wqkvb, 0, [[384, NDC], [1, 256]]),
                in_=_dram3(wq, 0, 128, NDC, 128, 256),
            )
            nc.gpsimd.dma_start(
                out=_sb(wqkvb, 256, [[384, NDC], [1, 128]]),
                in_=_dram3(wkv, 0, 128, NDC, 128, 128),
            )
            # cos/sin token-major, replicated x4 (one replica per local q-head)
            cs4x = const.tile([128, 16 * 128], BF16)
            sn4x = const.tile([128, 16 * 128], BF16)
            for h in range(4):
                nc.gpsimd.dma_start(
                    out=_sb(cs4x, h * 32, [[128, 16], [1, 32]]),
                    in_=_dram3(fcos, 0, 128, 16, 128, 32),
                )
                nc.gpsimd.dma_start(
                    out=_sb(sn4x, h * 32, [[128, 16], [1, 32]]),
                    in_=_dram3(fsin, 0, 128, 16, 128, 32),
                )

            # diagonal triangle masks: triA[k,q]=1 iff q>=k; triB=[0 | triA]
            triA = const.tile([128, 128], BF16)
            nc.gpsimd.memset(triA[:], 0.0)
            nc.gpsimd.affine_select(
                out=triA[:],
                in_=triA[:],
                compare_op=ALU.is_gt,
                fill=1.0,
                base=0,
                pattern=[[-1, 128]],
                channel_multiplier=1,
            )
            triB = const.tile([128, 256], BF16)
            nc.gpsimd.memset(triB[:], 0.0)
            nc.gpsimd.affine_select(
                out=triB[:, 128:256],
                in_=triB[:, 128:256],
                compare_op=ALU.is_gt,
                fill=1.0,
                base=0,
                pattern=[[-1, 128]],
                channel_multiplier=1,
            )

            # ---- persistent activations ----
            QT = [big.tile([128, S], BF16, name=f"QT{i}") for i in range(2)]
            KT2 = big.tile([128, S], BF16)
            Vext = big.tile([128, 16 * 65], BF16)
            nc.vector.memset(Vext[:], 1.0)
            AT = [big.tile([128, S], BF16, name=f"AT{i}") for i in range(2)]

            wob = wpool.tile([128, NDC * 256], BF16)

            ag_in = [dram.tile([256, SC], BF16, name=f"ag_in{i}") for i in range(3)]
            ag_out = [
                dram.tile([CORES * 256, SC], BF16, addr_space="Shared", name=f"ag_out{i}")
                for i in range(3)
            ]
            ag_in3 = [dram.tile([128, SC], BF16, name=f"ag_in3{m}") for m in range(2)]
            ag_out3 = [
                dram.tile([CORES * 128, SC], BF16, addr_space="Shared", name=f"ag_out3{m}")
                for m in range(2)
            ]

            xbfs = {0: xbf0}

            def emit_E(k):
                """Output projection for query chunk k (needs ag_out[k])."""
                ats = agt.tile([128, 16 * SC], BF16, name="ats", tag="ats")
                if k < 3:
                    nc.sync.dma_start(
                        out=ats[:], in_=_dram3(ag_out[k], 0, 128, 16, 128, SC)
                    )
                    rhs = [(e, ats, e) for e in range(16)]
                else:
                    nc.sync.dma_start(
                        out=ats[:, 0 : 8 * SC],
                        in_=_dram3(ag_out3[0], 0, 128, 8, 128, SC),
                    )
                    nc.sync.dma_start(
                        out=ats[:, 8 * SC : 16 * SC],
                        in_=_dram3(ag_out3[1], 0, 128, 8, 128, SC),
                    )
                    rhs = [(2 * r, ats, r) for r in range(8)] + [
                        (2 * r + 1, ats, 8 + r) for r in range(8)
                    ]
                for mb in range(2):
                    po = pvp.tile([128, SC], F32, name="po", tag="pv")
                    for i, (e, t, b) in enumerate(rhs):
                        nc.tensor.matmul(
                            po[:],
                            wob[:, e * 256 + mb * 128 : e * 256 + mb * 128 + 128],
                            t[:, b * SC : b * SC + SC],
                            start=(i == 0),
                            stop=(i == 15),
                        )
                    oT = otp.tile([128, SC], F32, name="oT")
                    nc.vector.tensor_copy(oT[:], po[:])
                    nc.sync.dma_start(
                        out=out[mb * 128 : mb * 128 + 128, k * SC : k * SC + SC],
                        in_=oT[:],
                    )

            for sc in range(NSC):
                # prefetch next x chunk (SWDGE cast f32->bf16)
                if sc + 1 < NSC:
                    xnx = xp.tile([128, TB * D], BF16, name="xbf")
                    nc.gpsimd.dma_start(
                        out=xnx[:], in_=_dram3(x, (sc + 1) * SC, 128, TB, 128, D)
                    )
                    xbfs[sc + 1] = xnx
                if sc == 0:
                    nc.gpsimd.dma_start(
                        out=wob[:], in_=_dram3(wo, 0, 128, NDC, 128, 256)
                    )
                xbf = xbfs.pop(sc)

                # x^T via XBAR DMA transpose: [128 tok, D] -> [128 d, 16 dc, 128 tok]
                xTt = []
                for tb in range(TB):
                    t = xtp.tile([128, NDC * 128], BF16, name="xT")
                    nc.sync.dma_start(
                        out=_sb(t, 0, [[128, NDC], [1, 128]]),
                        in_=xbf[:, tb * D : (tb + 1) * D],
                        transpose=True,
                    )
                    xTt.append(t)

                # projections + rope, token-major per 128-token block
                for tb in range(TB):
                    jblk = sc * TB + tb
                    qkv = pp.tile([128, 384], F32, name="qkv")
                    for dc in range(NDC):
                        nc.tensor.matmul(
                            qkv[:],
                            xTt[tb][:, dc * 128 : dc * 128 + 128],
                            wqkvb[:, dc * 384 : dc * 384 + 384],
                            start=(dc == 0),
                            stop=(dc == NDC - 1),
                        )
                    cosb = _sb(cs4x, jblk * 128, [[32, 4], [1, 32]])
                    sinb = _sb(sn4x, jblk * 128, [[32, 4], [1, 32]])
                    cos1 = _sb(cs4x, jblk * 128, [[1, 32]])
                    sin1 = _sb(sn4x, jblk * 128, [[1, 32]])
                    # stage PSUM -> SBUF bf16 once (GpSimd cannot read PSUM)
                    qkvb = qkb.tile([128, 384], BF16, name="qkvb")
                    nc.vector.tensor_copy(qkvb[:], qkv[:])
                    # Q rope: all 4 local heads at once (stride-2 pairs)
                    qE = _sb(qkvb, 0, [[64, 4], [2, 32]])
                    qO = _sb(qkvb, 1, [[64, 4], [2, 32]])
                    qst = qs.tile([128, 256], BF16, name="qst")
                    t1 = rp.tile([128, 128], BF16, name="t1")
                    t2 = rp.tile([128, 128], BF16, name="t2")
                    t3 = rp.tile([128, 128], BF16, name="t3")
                    t4 = rp.tile([128, 128], BF16, name="t4")
                    v1 = _sb(t1, 0, [[32, 4], [1, 32]])
                    v2 = _sb(t2, 0, [[32, 4], [1, 32]])
                    v3 = _sb(t3, 0, [[32, 4], [1, 32]])
                    v4 = _sb(t4, 0, [[32, 4], [1, 32]])
                    outE = _sb(qst, 0, [[64, 4], [2, 32]])
                    outO = _sb(qst, 1, [[64, 4], [2, 32]])
                    nc.vector.tensor_mul(v1, qE, cosb)
                    nc.vector.tensor_mul(v2, qO, sinb)
                    nc.vector.tensor_sub(outE, v1, v2)
                    nc.gpsimd.tensor_mul(v3, qE, sinb)
                    nc.gpsimd.tensor_mul(v4, qO, cosb)
                    nc.gpsimd.tensor_add(outO, v3, v4)
                    # K rope (cols 256:320)
                    kE = _sb(qkvb, 256, [[2, 32]])
                    kO = _sb(qkvb, 257, [[2, 32]])
                    kst = kvs.tile([128, 128], BF16, name="kst")
                    k1 = rp.tile([128, 32], BF16, name="k1", tag="k")
                    k2 = rp.tile([128, 32], BF16, name="k2", tag="k")
                    k3 = rp.tile([128, 32], BF16, name="k3", tag="k")
                    k4 = rp.tile([128, 32], BF16, name="k4", tag="k")
                    koutE = _sb(kst, 0, [[2, 32]])
                    koutO = _sb(kst, 1, [[2, 32]])
                    nc.vector.tensor_mul(k1[:], kE, cos1)
                    nc.vector.tensor_mul(k2[:], kO, sin1)
                    nc.vector.tensor_sub(koutE, k1[:], k2[:])
                    nc.gpsimd.tensor_mul(k3[:], kE, sin1)
                    nc.gpsimd.tensor_mul(k4[:], kO, cos1)
                    nc.gpsimd.tensor_add(koutO, k3[:], k4[:])
                    # V straight to key-major tile (ones col untouched)
                    nc.gpsimd.tensor_copy(
                        Vext[:, jblk * 65 : jblk * 65 + 64], qkvb[:, 320:384]
                    )
                    # transposes: Q (per head-pair) and K
                    for mb in range(2):
                        nc.sync.dma_start(
                            out=QT[mb][:, jblk * 128 : jblk * 128 + 128],
                            in_=qst[:, mb * 128 : mb * 128 + 128],
                            transpose=True,
                        )
                    nc.sync.dma_start(
                        out=KT2[:, jblk * 128 : jblk * 128 + 128],
                        in_=kst[:],
                        transpose=True,
                    )
                # duplicate K rows for row-group packed scores
                nc.gpsimd.tensor_copy(
                    KT2[64:128, sc * SC : sc * SC + SC],
                    KT2[0:64, sc * SC : sc * SC + SC],
                )

                # ============ attention for query chunk sc ============
                nblk = 4 * sc + 4
                for mb in range(2):
                    pv = [
                        pvp.tile([128, SC], F32, name="pv", tag="pv")
                        for _ in range(2)
                    ]
                    for jj in range(0, nblk, 2):
                        t0 = jj - 4 * sc
                        c0 = max(t0, 0) * 128
                        W = SC - c0
                        ps = [
                            sp.tile([128, 2 * SC], F32, name="ps", tag="ps")
                            for _ in range(2)
                        ]
                        for dj in range(2):
                            j = jj + dj
                            for lh in range(2):
                                r0 = 64 * lh
                                nc.tensor.matmul(
                                    ps[lh][:, dj * SC + c0 : dj * SC + SC],
                                    KT2[r0 : r0 + 64, j * 128 : j * 128 + 128],
                                    QT[mb][r0 : r0 + 64, sc * SC + c0 : sc * SC + SC],
                                    start=True,
                                    stop=True,
                                )
                        pt = [
                            ptp.tile([128, 2 * SC], BF16, name="pt", tag="pt")
                            for _ in range(2)
                        ]
                        for lh in range(2):
                            nc.scalar.activation(
                                _sb(pt[lh], c0, [[SC, 2], [1, W]]),
                                _sb(ps[lh], c0, [[SC, 2], [1, W]]),
                                AF.Exp,
                                scale=0.125,
                            )
                        if t0 >= 0:
                            for lh in range(2):
                                nc.vector.tensor_mul(
                                    pt[lh][:, c0 : c0 + 128],
                                    pt[lh][:, c0 : c0 + 128],
                                    triA[:],
                                )
                                nc.vector.tensor_mul(
                                    pt[lh][:, SC + c0 : SC + c0 + 256],
                                    pt[lh][:, SC + c0 : SC + c0 + 256],
                                    triB[:],
                                )
                        for dj in range(2):
                            j = jj + dj
                            for lh in range(2):
                                nc.tensor.matmul(
                                    pv[lh][0:65, c0:SC],
                                    Vext[:, j * 65 : j * 65 + 65],
                                    pt[lh][:, dj * SC + c0 : dj * SC + SC],
                                    start=(j == 0),
                                    stop=(j == nblk - 1),
                                )
                    # normalize: recip -> broadcast -> scale
                    for lh in range(2):
                        r0 = 64 * lh
                        den = npool.tile([1, SC], F32, name="den", tag="den")
                        nc.vector.tensor_copy(den[:], pv[lh][64:65, :])
                        rf = npool.tile([1, SC], F32, name="rf", tag="rf")
                        nc.vector.reciprocal_approx_fast(rf[:], den[:])
                        rb = npool.tile([1, SC], BF16, name="rb", tag="rb")
                        nc.vector.tensor_copy(rb[:], rf[:])
                        rbc = npool.tile([64, SC], BF16, name="rbc", tag="rbc")
                        nc.gpsimd.partition_broadcast(rbc[:], rb[:])
                        nc.vector.tensor_mul(
                            AT[mb][r0 : r0 + 64, sc * SC : sc * SC + SC],
                            pv[lh][0:64, :],
                            rbc[:],
                        )
                    if sc == 3:
                        nc.sync.dma_start(
                            out=ag_in3[mb][:], in_=AT[mb][:, 3 * SC : 4 * SC]
                        )
                        nc.gpsimd.collective_compute(
                            "AllGather",
                            ALU.bypass,
                            ins=[ag_in3[mb].opt()],
                            outs=[ag_out3[mb].opt()],
                            replica_groups=[list(range(CORES))],
                        )
                if sc < 3:
                    nc.sync.dma_start(
                        out=ag_in[sc][0:128, :], in_=AT[0][:, sc * SC : sc * SC + SC]
                    )
                    nc.sync.dma_start(
                        out=ag_in[sc][128:256, :], in_=AT[1][:, sc * SC : sc * SC + SC]
                    )
                    nc.gpsimd.collective_compute(
                        "AllGather",
                        ALU.bypass,
                        ins=[ag_in[sc].opt()],
                        outs=[ag_out[sc].opt()],
                        replica_groups=[list(range(CORES))],
                    )
                if sc >= 1:
                    emit_E(sc - 1)
            emit_E(3)
            if dbg:
                nc.sync.dma_start(out=d_qt[:, :], in_=QT[0][:])
                nc.sync.dma_start(out=d_kt[:, :], in_=KT2[:])
                nc.sync.dma_start(out=d_vx[:, :], in_=Vext[:])
                nc.sync.dma_start(out=d_at[:, :], in_=AT[0][:])

    nc.compile()
    _NC_CACHE["nc"] = nc
    return nc


def _shard_inputs(x, freqs_cos, freqs_sin, mask, wq, wk, wv, wo):
    x2 = np.ascontiguousarray(x.reshape(S, D), dtype=np.float32)
    fc = np.ascontiguousarray(freqs_cos, dtype=np.float32)
    fs = np.ascontiguousarray(freqs_sin, dtype=np.float32)
    in_maps = []
    for c in range(CORES):
        g = c // 2
        wq_c = np.ascontiguousarray(wq[:, 256 * c : 256 * c + 256], dtype=np.float32)
        wkv_c = np.ascontiguousarray(
            np.concatenate(
                [wk[:, HD * g : HD * g + HD], wv[:, HD * g : HD * g + HD]], axis=1
            ),
            dtype=np.float32,
        )
        wo_c = np.ascontiguousarray(wo[:, 256 * c : 256 * c + 256], dtype=np.float32)
        in_maps.append(
            {"x": x2, "wq": wq_c, "wkv": wkv_c, "wo": wo_c, "fcos": fc, "fsin": fs}
        )
    return in_maps


def kernel(x, freqs_cos, freqs_sin, mask, wq, wk, wv, wo, _trace=False):
    nc = build()
    in_maps = _shard_inputs(x, freqs_cos, freqs_sin, mask, wq, wk, wv, wo)
    res = bass_utils.run_bass_kernel_spmd(
        nc, in_maps, core_ids=list(range(CORES)), trace=_trace
    )
    outp = np.empty((S, D), dtype=np.float32)
    for c in range(CORES):
        outp[:, 256 * c : 256 * c + 256] = res.results[c]["out"].T
    if _trace:
        kernel._last_exec_time_ns = res.exec_time_ns
        kernel._last_results = res
    return outp.reshape(1, S, D)


# revision 26
# speedup vs baseline: 1.1108x; 1.1108x over previous
"""GQA attention (B=1, S=2048, D=2048, H=32, KV=4, HD=64) on 8 TRN2 NeuronCores.

Sharding: tensor-parallel over heads. Core c owns q-heads [4c, 4c+4) and kv-head
c//2 (all four q-heads of a core share one kv head).

Per-core pipeline (single fused loop over 512-token chunks):
  1. x streamed in S-chunks via SWDGE cast-DMA (f32->bf16, token-major), then
     XBAR DMA-transposed to x^T blocks (zero PE/ACT cost).
  2. Projections token-major: stationary x^T blocks, streaming [wq|wkv] (N=384),
     one PSUM bank per 128-token block.
  3. RoPE applied token-major (cos/sin per-partition tiles, natural (2i,2i+1)
     pairing - no weight permutation), split across DVE/GpSimd, reading PSUM
     directly; Q/K then DMA-transposed to Q^T/K^T; V copied straight into the
     key-major V tile (ones column for softmax denominators).
  4. Flash-style causal attention per (head-pair, query-chunk): scores^T via
     PE row-group packing (2 heads), exp on ScalarE over [128,2x512] tiles,
     matmul N-ranges restricted to the causal band at pair granularity,
     triangle masks only on diagonal strips, PV with ones-column denominator.
  5. Normalize via DVE reciprocal + GpSimd partition_broadcast, chunked
     AllGather (last chunk split per head-pair to shorten the tail), output
     projection interleaved one chunk behind attention.
Returns out^T [256, 2048] per core; the host transposes/concatenates.
"""

import os
from contextlib import ExitStack

import numpy as np

import concourse.bass as bass
import concourse.mybir as mybir
import concourse.tile as tile
from concourse import bacc
from concourse import bass_utils

F32 = mybir.dt.float32
BF16 = mybir.dt.bfloat16
AF = mybir.ActivationFunctionType
ALU = mybir.AluOpType

S = 2048
D = 2048
HD = 64
CORES = 8
SC = 512
NSC = S // SC
NDC = D // 128
TB = 4  # 128-token blocks per chunk

_NC_CACHE = {}


def _dram3(t, row0, nrow_p, nblk, blk_stride, ncol, col0=0):
    """AP over DRAM tensor/AP t: [partition p, block b, col f] ->
    t[row0 + b*blk_stride + p, col0 + f], p<nrow_p, b<nblk, f<ncol."""
    if isinstance(t, bass.AP):
        handle, off0, row_pitch = t.tensor, t.offset, t.tensor.shape[1]
    else:
        handle, off0, row_pitch = t, 0, t.shape[1]
    return bass.AP(
        handle,
        off0 + row0 * row_pitch + col0,
        [[row_pitch, nrow_p], [blk_stride * row_pitch, nblk], [1, ncol]],
    )


def _sb(t, off, dims):
    """Multi-dim free AP over an SBUF/PSUM tile at element offset off."""
    return bass.AP(t.tensor, t.offset + off, [t.ap[0], *dims])


def build():
    if "nc" in _NC_CACHE:
        return _NC_CACHE["nc"]
    nc = bacc.Bacc(None, target_bir_lowering=False, debug=False)

    x = nc.declare_dram_parameter("x", [S, D], F32, isOutput=False)
    wq = nc.declare_dram_parameter("wq", [D, 256], F32, isOutput=False)
    wkv = nc.declare_dram_parameter("wkv", [D, 128], F32, isOutput=False)
    wo = nc.declare_dram_parameter("wo", [D, 256], F32, isOutput=False)
    fcos = nc.declare_dram_parameter("fcos", [S, 32], F32, isOutput=False)
    fsin = nc.declare_dram_parameter("fsin", [S, 32], F32, isOutput=False)
    out = nc.declare_dram_parameter("out", [256, S], F32, isOutput=True)
    dbg = os.environ.get("KDBG") == "1"
    if dbg:
        d_qt = nc.declare_dram_parameter("d_qt", [128, S], BF16, isOutput=True)
        d_kt = nc.declare_dram_parameter("d_kt", [128, S], BF16, isOutput=True)
        d_vx = nc.declare_dram_parameter("d_vx", [128, 16 * 65], BF16, isOutput=True)
        d_at = nc.declare_dram_parameter("d_at", [128, S], BF16, isOutput=True)

    with tile.TileContext(nc) as tc:
        with ExitStack() as stack:
            ent = stack.enter_context
            const = ent(tc.tile_pool(name="const", bufs=1))
            wpool = ent(tc.tile_pool(name="wpool", bufs=1))
            big = ent(tc.tile_pool(name="big", bufs=1))
            xp = ent(tc.tile_pool(name="xp", bufs=2))
            xtp = ent(tc.tile_pool(name="xtp", bufs=5))
            qs = ent(tc.tile_pool(name="qs", bufs=3))
            qkb = ent(tc.tile_pool(name="qkb", bufs=3))
            rp = ent(tc.tile_pool(name="rp", bufs=6))
            ptp = ent(tc.tile_pool(name="ptp", bufs=4))
            npool = ent(tc.tile_pool(name="npool", bufs=2))
            agt = ent(tc.tile_pool(name="agt", bufs=2))
            otp = ent(tc.tile_pool(name="otp", bufs=2))
            pp = ent(tc.tile_pool(name="pp", bufs=2, space="PSUM"))
            sp = ent(tc.tile_pool(name="sp", bufs=2, space="PSUM"))
            pvp = ent(tc.tile_pool(name="pvp", bufs=2, space="PSUM"))
            dram = ent(tc.tile_pool(name="dram", bufs=1, space="DRAM"))
            # ---- critical path: x chunk0 block0, weights, cos/sin, rest of x
            xbf0 = xp.tile([128, TB * D], BF16, name="xbf")
            nc.gpsimd.dma_start(out=xbf0[:, 0:D], in_=x[0:128, :])
            # weights: [wq | wkv] interleaved per 128-row D-block
            wqkvb = wpool.tile([128, NDC * 384], BF16)
            nc.gpsimd.dma_start(
                out=_sb(wqkvb, 0, [[384, NDC], [1, 256]]),
                in_=_dram3(wq, 0, 128, NDC, 128, 256),
            )
            nc.gpsimd.dma_start(
                out=_sb(wqkvb, 256, [[384, NDC], [1, 128]]),
                in_=_dram3(wkv, 0, 128, NDC, 128, 128),
            )
            # cos/sin token-major (heads share via stride-0 APs)
            csb = const.tile([128, 16 * 32], BF16)
            snb = const.tile([128, 16 * 32], BF16)
            nc.gpsimd.dma_start(
                out=csb[:], in_=_dram3(fcos, 0, 128, 16, 128, 32)
            )
            nc.gpsimd.dma_start(
                out=snb[:], in_=_dram3(fsin, 0, 128, 16, 128, 32)
            )
            for tb in range(1, TB):
                nc.gpsimd.dma_start(
                    out=xbf0[:, tb * D : (tb + 1) * D],
                    in_=x[tb * 128 : tb * 128 + 128, :],
                )

            # diagonal triangle masks: triA[k,q]=1 iff q>=k; triB=[0 | triA]
            triA = const.tile([128, 128], BF16)
            nc.gpsimd.memset(triA[:], 0.0)
            nc.gpsimd.affine_select(
                out=triA[:],
                in_=triA[:],
                compare_op=ALU.is_gt,
                fill=1.0,
                base=0,
                pattern=[[-1, 128]],
                channel_multiplier=1,
            )
            triB = const.tile([128, 256], BF16)
            nc.gpsimd.memset(triB[:], 0.0)
            nc.gpsimd.affine_select(
                out=triB[:, 128:256],
                in_=triB[:, 128:256],
                compare_op=ALU.is_gt,
                fill=1.0,
                base=0,
                pattern=[[-1, 128]],
                channel_multiplier=1,
            )

            # ---- persistent activations ----
            # TA: [0,S)=Q^T mb0, [S,2S)=Q^T mb1, [2S,3S)=K^T (dup'd rows 64:128)
            TA = big.tile([128, 3 * S], BF16)
            Vext = big.tile([128, 16 * 65], BF16)
            nc.vector.memset(Vext[:], 1.0)
            AT = [big.tile([128, S], BF16, name=f"AT{i}") for i in range(2)]

            wob = wpool.tile([128, NDC * 256], BF16)

            ag_in = [dram.tile([256, SC], BF16, name=f"ag_in{i}") for i in range(3)]
            ag_out = [
                dram.tile([CORES * 256, SC], BF16, addr_space="Shared", name=f"ag_out{i}")
                for i in range(3)
            ]
            ag_in3 = [dram.tile([128, SC], BF16, name=f"ag_in3{m}") for m in range(2)]
            ag_out3 = [
                dram.tile([CORES * 128, SC], BF16, addr_space="Shared", name=f"ag_out3{m}")
                for m in range(2)
            ]

            xbfs = {0: xbf0}

            def emit_E(k):
                """Output projection for query chunk k (needs ag_out[k])."""
                ats = agt.tile([128, 16 * SC], BF16, name="ats", tag="ats")
                if k < 3:
                    nc.scalar.dma_start(
                        out=ats[:], in_=_dram3(ag_out[k], 0, 128, 16, 128, SC)
                    )
                    rhs = [(e, ats, e) for e in range(16)]
                else:
                    nc.scalar.dma_start(
                        out=ats[:, 0 : 8 * SC],
                        in_=_dram3(ag_out3[0], 0, 128, 8, 128, SC),
                    )
                    nc.scalar.dma_start(
                        out=ats[:, 8 * SC : 16 * SC],
                        in_=_dram3(ag_out3[1], 0, 128, 8, 128, SC),
                    )
                    rhs = [(2 * r, ats, r) for r in range(8)] + [
                        (2 * r + 1, ats, 8 + r) for r in range(8)
                    ]
                for mb in range(2):
                    po = pvp.tile([128, SC], F32, name="po", tag="pv")
                    for i, (e, t, b) in enumerate(rhs):
                        nc.tensor.matmul(
                            po[:],
                            wob[:, e * 256 + mb * 128 : e * 256 + mb * 128 + 128],
                            t[:, b * SC : b * SC + SC],
                            start=(i == 0),
                            stop=(i == 15),
                        )
                    oT = otp.tile([128, SC], F32, name="oT")
                    nc.vector.tensor_copy(oT[:], po[:])
                    nc.scalar.dma_start(
                        out=out[mb * 128 : mb * 128 + 128, k * SC : k * SC + SC],
                        in_=oT[:],
                    )

            for sc in range(NSC):
                # prefetch next x chunk (SWDGE cast f32->bf16)
                if sc + 1 < NSC:
                    xnx = xp.tile([128, TB * D], BF16, name="xbf")
                    nc.gpsimd.dma_start(
                        out=xnx[:], in_=_dram3(x, (sc + 1) * SC, 128, TB, 128, D)
                    )
                    xbfs[sc + 1] = xnx
                if sc == 0:
                    nc.gpsimd.dma_start(
                        out=wob[:], in_=_dram3(wo, 0, 128, NDC, 128, 256)
                    )
                xbf = xbfs.pop(sc)

                # x^T via XBAR DMA transpose: [128 tok, D] -> [128 d, 16 dc, 128 tok]
                xTt = []
                for tb in range(TB):
                    t = xtp.tile([128, NDC * 128], BF16, name="xT")
                    nc.sync.dma_start(
                        out=_sb(t, 0, [[128, NDC], [1, 128]]),
                        in_=xbf[:, tb * D : (tb + 1) * D],
                        transpose=True,
                    )
                    xTt.append(t)

                # projections + rope, token-major per 128-token block
                for tb in range(TB):
                    jblk = sc * TB + tb
                    qkv = pp.tile([128, 384], F32, name="qkv")
                    for dc in range(NDC):
                        nc.tensor.matmul(
                            qkv[:],
                            xTt[tb][:, dc * 128 : dc * 128 + 128],
                            wqkvb[:, dc * 384 : dc * 384 + 384],
                            start=(dc == 0),
                            stop=(dc == NDC - 1),
                        )
                    cosb = _sb(csb, jblk * 32, [[0, 4], [1, 32]])
                    sinb = _sb(snb, jblk * 32, [[0, 4], [1, 32]])
                    cos1 = _sb(csb, jblk * 32, [[1, 32]])
                    sin1 = _sb(snb, jblk * 32, [[1, 32]])
                    # stage PSUM -> SBUF bf16 once (GpSimd cannot read PSUM)
                    qkvb = qkb.tile([128, 384], BF16, name="qkvb")
                    nc.vector.tensor_copy(qkvb[:], qkv[:])
                    # Q rope (4 heads, stride-2 pairs) + K rope into one stage
                    qE = _sb(qkvb, 0, [[64, 4], [2, 32]])
                    qO = _sb(qkvb, 1, [[64, 4], [2, 32]])
                    stg = qs.tile([128, 384], BF16, name="stg")
                    t1 = rp.tile([128, 128], BF16, name="t1")
                    t2 = rp.tile([128, 128], BF16, name="t2")
                    t3 = rp.tile([128, 128], BF16, name="t3")
                    t4 = rp.tile([128, 128], BF16, name="t4")
                    v1 = _sb(t1, 0, [[32, 4], [1, 32]])
                    v2 = _sb(t2, 0, [[32, 4], [1, 32]])
                    v3 = _sb(t3, 0, [[32, 4], [1, 32]])
                    v4 = _sb(t4, 0, [[32, 4], [1, 32]])
                    outE = _sb(stg, 0, [[64, 4], [2, 32]])
                    outO = _sb(stg, 1, [[64, 4], [2, 32]])
                    nc.vector.tensor_mul(v1, qE, cosb)
                    nc.vector.tensor_mul(v2, qO, sinb)
                    nc.vector.tensor_sub(outE, v1, v2)
                    nc.gpsimd.tensor_mul(v3, qE, sinb)
                    nc.gpsimd.tensor_mul(v4, qO, cosb)
                    nc.gpsimd.tensor_add(outO, v3, v4)
                    # K rope (cols 256:320; 320:384 left as garbage)
                    kE = _sb(qkvb, 256, [[2, 32]])
                    kO = _sb(qkvb, 257, [[2, 32]])
                    k1 = rp.tile([128, 32], BF16, name="k1", tag="k")
                    k2 = rp.tile([128, 32], BF16, name="k2", tag="k")
                    k3 = rp.tile([128, 32], BF16, name="k3", tag="k")
                    k4 = rp.tile([128, 32], BF16, name="k4", tag="k")
                    koutE = _sb(stg, 256, [[2, 32]])
                    koutO = _sb(stg, 257, [[2, 32]])
                    nc.vector.tensor_mul(k1[:], kE, cos1)
                    nc.vector.tensor_mul(k2[:], kO, sin1)
                    nc.vector.tensor_sub(koutE, k1[:], k2[:])
                    nc.gpsimd.tensor_mul(k3[:], kE, sin1)
                    nc.gpsimd.tensor_mul(k4[:], kO, cos1)
                    nc.gpsimd.tensor_add(koutO, k3[:], k4[:])
                    # V straight to key-major tile (ones col untouched)
                    nc.gpsimd.tensor_copy(
                        Vext[:, jblk * 65 : jblk * 65 + 64], qkvb[:, 320:384]
                    )
                    # one XBAR transpose per token-block: Qmb0|Qmb1|K -> TA
                    nc.sync.dma_start(
                        out=_sb(TA, jblk * 128, [[S, 3], [1, 128]]),
                        in_=stg[:],
                        transpose=True,
                    )
                # duplicate K rows for row-group packed scores
                nc.gpsimd.tensor_copy(
                    TA[64:128, 2 * S + sc * SC : 2 * S + sc * SC + SC],
                    TA[0:64, 2 * S + sc * SC : 2 * S + sc * SC + SC],
                )

                # ============ attention for query chunk sc ============
                nblk = 4 * sc + 4
                for mb in range(2):
                    pv = [
                        pvp.tile([128, SC], F32, name="pv", tag="pv")
                        for _ in range(2)
                    ]
                    for jj in range(0, nblk, 2):
                        t0 = jj - 4 * sc
                        c0 = max(t0, 0) * 128
                        W = SC - c0
                        ps = [
                            sp.tile([128, 2 * SC], F32, name="ps", tag="ps")
                            for _ in range(2)
                        ]
                        for dj in range(2):
                            j = jj + dj
                            for lh in range(2):
                                r0 = 64 * lh
                                nc.tensor.matmul(
                                    ps[lh][:, dj * SC + c0 : dj * SC + SC],
                                    TA[r0 : r0 + 64, 2 * S + j * 128 : 2 * S + j * 128 + 128],
                                    TA[r0 : r0 + 64, mb * S + sc * SC + c0 : mb * S + sc * SC + SC],
                                    start=True,
                                    stop=True,
                                )
                        pt = [
                            ptp.tile([128, 2 * SC], BF16, name="pt", tag="pt")
                            for _ in range(2)
                        ]
                        for lh in range(2):
                            nc.scalar.activation(
                                _sb(pt[lh], c0, [[SC, 2], [1, W]]),
                                _sb(ps[lh], c0, [[SC, 2], [1, W]]),
                                AF.Exp,
                                scale=0.125,
                            )
                        if t0 >= 0:
                            for lh in range(2):
                                nc.vector.tensor_mul(
                                    pt[lh][:, c0 : c0 + 128],
                                    pt[lh][:, c0 : c0 + 128],
                                    triA[:],
                                )
                                nc.vector.tensor_mul(
                                    pt[lh][:, SC + c0 : SC + c0 + 256],
                                    pt[lh][:, SC + c0 : SC + c0 + 256],
                                    triB[:],
                                )
                        for dj in range(2):
                            j = jj + dj
                            for lh in range(2):
                                nc.tensor.matmul(
                                    pv[lh][0:65, c0:SC],
                                    Vext[:, j * 65 : j * 65 + 65],
                                    pt[lh][:, dj * SC + c0 : dj * SC + SC],
                                    start=(j == 0),
                                    stop=(j == nblk - 1),
                                )
                    # normalize: recip -> broadcast -> scale
                    for lh in range(2):
                        r0 = 64 * lh
                        den = npool.tile([1, SC], F32, name="den", tag="den")
                        nc.vector.tensor_copy(den[:], pv[lh][64:65, :])
                        rf = npool.tile([1, SC], F32, name="rf", tag="rf")
                        nc.vector.reciprocal_approx_fast(rf[:], den[:])
                        rb = npool.tile([1, SC], BF16, name="rb", tag="rb")
                        nc.vector.tensor_copy(rb[:], rf[:])
                        rbc = npool.tile([64, SC], BF16, name="rbc", tag="rbc")
                        nc.gpsimd.partition_broadcast(rbc[:], rb[:])
                        nc.vector.tensor_mul(
                            AT[mb][r0 : r0 + 64, sc * SC : sc * SC + SC],
                            pv[lh][0:64, :],
                            rbc[:],
                        )
                    if sc == 3:
                        nc.scalar.dma_start(
                            out=ag_in3[mb][:], in_=AT[mb][:, 3 * SC : 4 * SC]
                        )
                        nc.gpsimd.collective_compute(
                            "AllGather",
                            ALU.bypass,
                            ins=[ag_in3[mb].opt()],
                            outs=[ag_out3[mb].opt()],
                            replica_groups=[list(range(CORES))],
                        )
                if sc < 3:
                    nc.scalar.dma_start(
                        out=ag_in[sc][0:128, :], in_=AT[0][:, sc * SC : sc * SC + SC]
                    )
                    nc.scalar.dma_start(
                        out=ag_in[sc][128:256, :], in_=AT[1][:, sc * SC : sc * SC + SC]
                    )
                    nc.gpsimd.collective_compute(
                        "AllGather",
                        ALU.bypass,
                        ins=[ag_in[sc].opt()],
                        outs=[ag_out[sc].opt()],
                        replica_groups=[list(range(CORES))],
                    )
                if sc >= 1:
                    emit_E(sc - 1)
            emit_E(3)
            if dbg:
                nc.sync.dma_start(out=d_qt[:, :], in_=TA[:, 0:S])
                nc.sync.dma_start(out=d_kt[:, :], in_=TA[:, 2 * S : 3 * S])
                nc.sync.dma_start(out=d_vx[:, :], in_=Vext[:])
                nc.sync.dma_start(out=d_at[:, :], in_=AT[0][:])

    nc.compile()
    _NC_CACHE["nc"] = nc
    return nc


def _shard_inputs(x, freqs_cos, freqs_sin, mask, wq, wk, wv, wo):
    x2 = np.ascontiguousarray(x.reshape(S, D), dtype=np.float32)
    fc = np.ascontiguousarray(freqs_cos, dtype=np.float32)
    fs = np.ascontiguousarray(freqs_sin, dtype=np.float32)
    in_maps = []
    for c in range(CORES):
        g = c // 2
        wq_c = np.ascontiguousarray(wq[:, 256 * c : 256 * c + 256], dtype=np.float32)
        wkv_c = np.ascontiguousarray(
            np.concatenate(
                [wk[:, HD * g : HD * g + HD], wv[:, HD * g : HD * g + HD]], axis=1
            ),
            dtype=np.float32,
        )
        wo_c = np.ascontiguousarray(wo[:, 256 * c : 256 * c + 256], dtype=np.float32)
        in_maps.append(
            {"x": x2, "wq": wq_c, "wkv": wkv_c, "wo": wo_c, "fcos": fc, "fsin": fs}
        )
    return in_maps


def kernel(x, freqs_cos, freqs_sin, mask, wq, wk, wv, wo, _trace=False):
    nc = build()
    in_maps = _shard_inputs(x, freqs_cos, freqs_sin, mask, wq, wk, wv, wo)
    res = bass_utils.run_bass_kernel_spmd(
        nc, in_maps, core_ids=list(range(CORES)), trace=_trace
    )
    outp = np.empty((S, D), dtype=np.float32)
    for c in range(CORES):
        outp[:, 256 * c : 256 * c + 256] = res.results[c]["out"].T
    if _trace:
        kernel._last_exec_time_ns = res.exec_time_ns
        kernel._last_results = res
    return outp.reshape(1, S, D)
